# revision 19
# baseline (speedup 1.0000x reference)
"""MoE (GPT MLP, top-2, GShard capacity) kernel for 8 Trainium2 NeuronCores.

Strategy (expert-parallel, matching the sharding hint):
  - Host: fp32 gate (softmax + top-2 + GShard capacity positions), dispatch
    gather.  Routing is O(N*E) int/scalar work - negligible next to the FFN -
    and the capacity scan is inherently sequential, so it runs on host.
  - Device: 8 cores, core e owns expert e.  Each core runs the expert FFN
    y = gelu(disp @ w1 + b1) @ w2 over its cap=2048 dispatched token slots.
    All matmuls in bf16 (inputs rounded host-side; PSUM accumulates fp32),
    which runs at the full PE rate with cheap (hideable) LDWEIGHTS and
    halves HBM traffic.  Weights stream exactly once: the whole 2048-token
    h activation stays resident in SBUF as bf16 (128 KB/partition).
  - Host: combine (gather + gate-weighted sum) + b2.

Self-contained: hardcodes B=4, S=2048, D=1024, H=4096, E=8, K=2, cap=2048.
"""

import sys

sys.path.insert(0, "/opt/trn_rl_repo")

import numpy as np
import ml_dtypes

BF16 = ml_dtypes.bfloat16

B, S, D, H, E = 4, 2048, 1024, 4096, 8
K = 2
N_TOK = B * S            # 8192
CAP = (K * N_TOK) // E   # 2048 (capacity factor 1.0)
EPS = 1e-9
P = 128                  # SBUF partitions

_NC_CACHE = {}
_ROUTE_STATE = None


# --------------------------------------------------------------------------
# Host routing (replicates reference.py's gate exactly, in numpy fp32)
# --------------------------------------------------------------------------

def _route(xt, wg):
    """xt: [N, D] fp32, wg: [D, E] fp32 ->
    gidx [N,K] int, gvals [N,K] fp32 (keep-masked), pos [N,K] int, keep [N,K]"""
    logits = xt @ wg                                   # [N, E] fp32
    m = logits.max(axis=-1, keepdims=True)
    ex = np.exp(logits - m)
    scores = ex / ex.sum(axis=-1, keepdims=True)
    order = np.argsort(-scores, axis=1, kind="stable")  # jax top_k tie rule
    gidx = order[:, :K]                                 # [N, K]
    gvals = np.take_along_axis(scores, gidx, axis=1)
    gvals = gvals / np.clip(gvals.sum(-1, keepdims=True), EPS, None)

    n = xt.shape[0]
    offset = np.zeros(E, np.int64)
    pos = np.zeros((n, K), np.int64)
    keep = np.zeros((n, K), bool)
    rows = np.arange(n)
    for kk in range(K):
        ek = gidx[:, kk]
        oh = np.zeros((n, E), np.int64)
        oh[rows, ek] = 1
        loc = np.cumsum(oh, axis=0) - 1 + offset[None, :]
        offset = offset + oh.sum(axis=0)
        p = loc[rows, ek]
        kmask = p < CAP
        pos[:, kk] = np.where(kmask, p, 0)
        keep[:, kk] = kmask
    gvals = (gvals * keep).astype(np.float32)
    return gidx, gvals, pos, keep


# --------------------------------------------------------------------------
# Device kernel builder (one expert FFN per core, SPMD)
# --------------------------------------------------------------------------

def _build_nc(d, h, ntok, debug=False, act="Gelu", reps=1):
    """Expert FFN: y[ntok, d] = gelu(x[ntok, d] @ w1[d, h] + b1[h]) @ w2[h, d].

    Single-pass over weights; all matmul operands bf16; h resident in SBUF.

    Phase A (per h-tile m of 128): for each 512-token chunk tc, accumulate
    over the 8 D-tiles into one PSUM bank, then gelu(+b1) into the resident
    bf16 h[m] tile.
    Phase B (per output D-tile dt): for each 512-token chunk, accumulate over
    the 32 h-tiles into one PSUM bank, copy to bf16, DMA out as y^T.

    Device inputs (pre-laid-out on host for contiguous DMA):
      xt  : [d/P, P, ntok]        bf16  x^T tiles (partition = D)
      w1t : [h/P, P, d/P, P]      bf16  [m][dp][dt][hf] (lhsT layout)
      w2t : [d/P, P, h/P, P]      bf16  [dt][hp][m][df] (lhsT layout)
      b1t : [P, h/P]              f32   b1 transposed
    Output:
      yt  : [d/P, P, ntok]        bf16  y^T tiles (partition = D)
    """
    from concourse import bacc, mybir, tile

    dt_n = d // P            # 8  D tiles
    mt_n = h // P            # 32 H tiles
    tc_n = ntok // 512       # 4  512-token chunks
    assert ntok % 512 == 0

    f32 = mybir.dt.float32
    bf16 = mybir.dt.bfloat16
    actf = getattr(mybir.ActivationFunctionType, act)

    nc = bacc.Bacc("TRN2", target_bir_lowering=False, debug=debug,
                   enable_asserts=False, num_devices=1)

    xt_d = nc.dram_tensor("xt", [dt_n, P, ntok], bf16, kind="ExternalInput")
    w1_d = nc.dram_tensor("w1t", [mt_n, P, dt_n, P], bf16, kind="ExternalInput")
    w2_d = nc.dram_tensor("w2t", [dt_n, P, mt_n, P], bf16, kind="ExternalInput")
    b1_d = nc.dram_tensor("b1t", [P, mt_n], f32, kind="ExternalInput")
    y_d = nc.dram_tensor("yt", [dt_n, P, ntok], bf16, kind="ExternalOutput")

    with tile.TileContext(nc) as tc:
        with (
            tc.tile_pool(name="xpool", bufs=dt_n) as xpool,
            tc.tile_pool(name="cpool", bufs=2) as cpool,
            tc.tile_pool(name="w1pool", bufs=4) as w1pool,
            tc.tile_pool(name="hpool", bufs=mt_n + 2) as hpool,
            tc.tile_pool(name="w2pool", bufs=2) as w2pool,
            tc.tile_pool(name="ypool", bufs=4) as ypool,
            tc.tile_pool(name="psA", bufs=4, space="PSUM") as psA,
            tc.tile_pool(name="psB", bufs=3, space="PSUM") as psB,
            tc.tile_pool(name="psW", bufs=1, space="PSUM") as psW,
        ):
            b1_t = cpool.tile([P, mt_n], f32)
            nc.sync.dma_start(b1_t[:], b1_d[:])

            def prefetch_w1x(rep):
                """First w1 tiles + all x tiles for a rep.  Emitted before
                the previous rep's phase B so the scalar-queue issues land
                ahead of that rep's y stores (no rep-boundary input stall).
                x is split across both hwdge queues for startup bandwidth."""
                w1_pre = {}
                w1m = w1pool.tile([P, dt_n, P], bf16, tag="w1m",
                                  name=f"w1m_{rep}_0")
                nc.scalar.dma_start(w1m[:], w1_d[0])
                w1_pre[0] = w1m
                xg = [None] * dt_n
                for dti in range(dt_n):
                    xg_t = xpool.tile([P, ntok], bf16, tag="xg",
                                      name=f"xg_{rep}_{dti}")
                    eng = nc.sync if dti % 2 == 0 else nc.scalar
                    eng.dma_start(xg_t[:], xt_d[dti])
                    xg[dti] = xg_t
                for m in range(1, 4):
                    w1m = w1pool.tile([P, dt_n, P], bf16, tag="w1m",
                                      name=f"w1m_{rep}_{m}")
                    nc.scalar.dma_start(w1m[:], w1_d[m])
                    w1_pre[m] = w1m
                return w1_pre, xg

            def phase_a(rep, w1_pre, xg):
                """h[m] = gelu(sum_dt w1[dt][m].T @ x[dt] + b1[m])"""
                hs = []
                for m in range(mt_n):
                    if m in w1_pre:
                        w1m = w1_pre.pop(m)
                    else:
                        w1m = w1pool.tile([P, dt_n, P], bf16, tag="w1m",
                                          name=f"w1m_{rep}_{m}")
                        nc.scalar.dma_start(w1m[:], w1_d[m])
                    hm = hpool.tile([P, ntok], bf16, tag="hm",
                                    name=f"hm_{rep}_{m}")
                    for tci in range(tc_n):
                        c0, c1 = tci * 512, (tci + 1) * 512
                        ps = psA.tile([P, 512], f32, tag="psA")
                        for dti in range(dt_n):
                            nc.tensor.matmul(
                                ps[:],
                                w1m[:, dti, :],
                                xg[dti][:, c0:c1],
                                start=(dti == 0),
                                stop=(dti == dt_n - 1),
                            )
                        nc.scalar.activation(
                            hm[:, c0:c1], ps[:], actf,
                            bias=b1_t[:, m:m + 1], scale=1.0,
                        )
                    hs.append(hm)
                return hs

            def phase_b(rep, w2_pre, hs):
                """y^T[dt] = sum_m w2[dt][m].T @ h[m]"""
                for dti in range(dt_n):
                    if dti in w2_pre:
                        w2m = w2_pre.pop(dti)
                    else:
                        w2m = w2pool.tile([P, mt_n, P], bf16, tag="w2m",
                                          name=f"w2m_{rep}_{dti}")
                        nc.scalar.dma_start(w2m[:], w2_d[dti])
                    for tci in range(tc_n):
                        c0, c1 = tci * 512, (tci + 1) * 512
                        ps = psB.tile([P, 512], f32, tag="psB")
                        for m in range(mt_n):
                            nc.tensor.matmul(
                                ps[:],
                                w2m[:, m, :],
                                hs[m][:, c0:c1],
                                start=(m == 0),
                                stop=(m == mt_n - 1),
                            )
                        yt = ypool.tile([P, 512], bf16, tag="yt")
                        nc.vector.tensor_copy(yt[:], ps[:])
                        nc.scalar.dma_start(y_d[dti][:, c0:c1], yt[:])

            def prefetch_w2(rep):
                """First w2 tiles so phase B's first chains don't wait on a
                DMA issued after phase A's last gelu."""
                w2_pre = {}
                for dti in range(2):
                    w2m = w2pool.tile([P, mt_n, P], bf16, tag="w2m",
                                      name=f"w2m_{rep}_{dti}")
                    nc.scalar.dma_start(w2m[:], w2_d[dti])
                    w2_pre[dti] = w2m
                return w2_pre

            # PE p-state warmup: harmless 128-row matmuls on a zeroed tile
            # executed while the first x tiles stream in, so the real
            # matmuls start at full clock (ramp needs ~3us of busy PE).
            warm = cpool.tile([P, P], bf16, name="warm")
            nc.vector.memset(warm[:], 0)
            wps = psW.tile([P, P], f32, tag="warm")
            for _ in range(48):
                nc.tensor.matmul(wps[:], warm[:], warm[:], start=True,
                                 stop=True)

            w1_pre, xg = prefetch_w1x(0)
            w2_pre = prefetch_w2(0)
            for rep in range(reps):
                hs = phase_a(rep, w1_pre, xg)
                if rep + 1 < reps:
                    w1_pre, xg = prefetch_w1x(rep + 1)
                phase_b(rep, w2_pre, hs)
                if rep + 1 < reps:
                    w2_pre = prefetch_w2(rep + 1)

    nc.compile()
    return nc


def _get_nc(d, h, ntok, debug=False, reps=1):
    key = (d, h, ntok, debug, reps)
    if key not in _NC_CACHE:
        _NC_CACHE[key] = _build_nc(d, h, ntok, debug, reps=reps)
    return _NC_CACHE[key]


# --------------------------------------------------------------------------
# Host-side input layout per core
# --------------------------------------------------------------------------

def _core_inputs(disp_e, w1_e, w2_e, b1_e):
    """disp_e: [CAP, D], w1_e: [D, H], w2_e: [H, D], b1_e: [H] fp32."""
    xt = np.ascontiguousarray(disp_e.T.reshape(D // P, P, CAP)).astype(BF16)
    w1t = np.ascontiguousarray(
        w1_e.reshape(D // P, P, H // P, P).transpose(2, 1, 0, 3)).astype(BF16)
    w2t = np.ascontiguousarray(
        w2_e.reshape(H // P, P, D // P, P).transpose(2, 1, 0, 3)).astype(BF16)
    b1t = np.ascontiguousarray(b1_e.reshape(H // P, P).T)
    return {"xt": xt, "w1t": w1t, "w2t": w2t, "b1t": b1t}


def _get_runner(nc, n_cores):
    """Cached PJRT executable for an SPMD bass program (axon path of
    run_bass_kernel_spmd, with the jitted callable kept warm across calls)."""
    key = id(nc)
    if key in _NC_CACHE:
        return _NC_CACHE[key]

    import jax
    from jax.sharding import Mesh, PartitionSpec
    from jax.experimental.shard_map import shard_map
    from concourse import mybir
    from concourse.bass2jax import (_bass_exec_p, install_neuronx_cc_hook,
                                    partition_id_tensor)

    install_neuronx_cc_hook()

    partition_name = (nc.partition_id_tensor.name
                      if nc.partition_id_tensor else None)
    in_names, out_names, out_avals = [], [], []
    for alloc in nc.m.functions[0].allocations:
        if not isinstance(alloc, mybir.MemoryLocationSet):
            continue
        name = alloc.memorylocations[0].name
        if alloc.kind == "ExternalInput":
            if name != partition_name:
                in_names.append(name)
        elif alloc.kind == "ExternalOutput":
            out_names.append(name)
            shape = tuple(alloc.tensor_shape)
            out_avals.append(jax.core.ShapedArray(shape, mybir.dt.np(alloc.dtype)))
    n_params = len(in_names)
    n_outs = len(out_avals)
    in_names = in_names + out_names
    if partition_name is not None:
        in_names.append(partition_name)
    donate = tuple(range(n_params, n_params + n_outs))

    def _body(*args):
        operands = list(args)
        if partition_name is not None:
            operands.append(partition_id_tensor())
        outs = _bass_exec_p.bind(
            *operands,
            out_avals=tuple(out_avals),
            in_names=tuple(in_names),
            out_names=tuple(out_names),
            lowering_input_output_aliases=(),
            sim_require_finite=True,
            sim_require_nnan=True,
            nc=nc,
        )
        return tuple(outs)

    devices = jax.devices()[:n_cores]
    mesh = Mesh(np.asarray(devices), ("core",))
    in_specs = (PartitionSpec("core"),) * (n_params + n_outs)
    out_specs = (PartitionSpec("core"),) * n_outs
    sharded = jax.jit(
        shard_map(_body, mesh=mesh, in_specs=in_specs, out_specs=out_specs,
                  check_rep=False),
        donate_argnums=donate, keep_unused=True,
    )

    def run(in_maps, reps=1, time_reps=False):
        import time as _time
        concat_in = [
            np.concatenate([np.asarray(m[in_names[i]]) for m in in_maps], axis=0)
            for i in range(n_params)
        ]
        concat_in = [jax.device_put(a) for a in concat_in]
        zero_sets = []
        for _ in range(reps):
            zero_sets.append([
                jax.device_put(np.zeros((n_cores * av.shape[0], *av.shape[1:]),
                                        av.dtype))
                for av in out_avals
            ])
        for zs in zero_sets:
            for z in zs:
                z.block_until_ready()
        for a in concat_in:
            a.block_until_ready()
        times = []
        out_arrs = None
        for r in range(reps):
            t0 = _time.perf_counter()
            out_arrs = sharded(*concat_in, *zero_sets[r])
            for o in out_arrs:
                o.block_until_ready()
            times.append(_time.perf_counter() - t0)
        results = [
            {name: np.asarray(out_arrs[i]).reshape(n_cores, *out_avals[i].shape)[c]
             for i, name in enumerate(out_names)}
            for c in range(n_cores)
        ]
        if time_reps:
            return results, times
        return results

    _NC_CACHE[key] = run
    return run


def prepare(inputs, reps=1):
    """Routing + dispatch + per-core device input layout.  Returns
    (in_maps, nc); routing state is stashed on the module for finish()."""
    x = np.asarray(inputs["x"], np.float32)
    wg = np.asarray(inputs["wg"], np.float32)
    w1 = np.asarray(inputs["w1"], np.float32)
    b1 = np.asarray(inputs["b1"], np.float32)
    w2 = np.asarray(inputs["w2"], np.float32)

    xt = x.reshape(N_TOK, D)
    gidx, gvals, pos, keep = _route(xt, wg)
    global _ROUTE_STATE
    _ROUTE_STATE = (gidx, gvals, pos)

    # dispatch: slots are unique per expert, so assignment == scatter-add
    disp = np.zeros((E, CAP, D), np.float32)
    for kk in range(K):
        tok = np.nonzero(keep[:, kk])[0]
        disp[gidx[tok, kk], pos[tok, kk]] = xt[tok]

    in_maps = [_core_inputs(disp[e], w1[e], w2[e], b1[e]) for e in range(E)]
    nc = _get_nc(D, H, CAP, reps=reps)
    return in_maps, nc


def finish(inputs, results):
    """Combine: out = sum_k gvals * (y[e, pos] + b2[e])."""
    b2 = np.asarray(inputs["b2"], np.float32)
    gidx, gvals, pos = _ROUTE_STATE
    # yt: [d/P, P, ntok] bf16 y^T -> y [ntok, d] fp32
    y_all = np.stack([
        np.asarray(r["yt"], dtype=np.float32).reshape(D, CAP).T
        for r in results])  # [E,CAP,D]
    e_flat = gidx.reshape(-1)
    p_flat = pos.reshape(-1)
    yk = y_all[e_flat, p_flat] + b2[e_flat]
    w = gvals.reshape(-1).astype(np.float32)
    out = (yk * w[:, None]).reshape(N_TOK, K, D).sum(axis=1)
    return out.reshape(B, S, D).astype(np.float32)


def kernel(x, wg, w1, b1, w2, b2):
    inputs = {"x": x, "wg": wg, "w1": w1, "b1": b1, "w2": w2, "b2": b2}
    in_maps, nc = prepare(inputs)
    run = _get_runner(nc, E)
    results = run(in_maps)
    return finish(inputs, results)


# --------------------------------------------------------------------------
# Benchmarking helpers (test.py only)
# --------------------------------------------------------------------------

def bench(x, wg, w1, b1, w2, b2, reps=10):
    """Returns (reps1_times, reps5_times) per-call wall seconds for timing."""
    inputs = {"x": x, "wg": wg, "w1": w1, "b1": b1, "w2": w2, "b2": b2}
    in_maps, nc1 = prepare(inputs, reps=1)
    run1 = _get_runner(nc1, E)
    _, t1 = run1(in_maps, reps=reps, time_reps=True)

    nc5 = _get_nc(D, H, CAP, reps=5)
    run5 = _get_runner(nc5, E)
    _, t5 = run5(in_maps, reps=reps, time_reps=True)
    return t1, t5


# revision 24
# speedup vs baseline: 1.0007x; 1.0007x over previous
"""MoE (GPT MLP, top-2, GShard capacity) kernel for 8 Trainium2 NeuronCores.

Strategy (expert-parallel, matching the sharding hint):
  - Host: fp32 gate (softmax + top-2 + GShard capacity positions), dispatch
    gather.  Routing is O(N*E) int/scalar work - negligible next to the FFN -
    and the capacity scan is inherently sequential, so it runs on host.
  - Device: 8 cores, core e owns expert e.  Each core runs the expert FFN
    y = gelu(disp @ w1 + b1) @ w2 over its cap=2048 dispatched token slots.
    All matmuls in bf16 (inputs rounded host-side; PSUM accumulates fp32),
    which runs at the full PE rate with cheap (hideable) LDWEIGHTS and
    halves HBM traffic.  Weights stream exactly once: the whole 2048-token
    h activation stays resident in SBUF as bf16 (128 KB/partition).
  - Host: combine (gather + gate-weighted sum) + b2.

Self-contained: hardcodes B=4, S=2048, D=1024, H=4096, E=8, K=2, cap=2048.
"""

import sys

sys.path.insert(0, "/opt/trn_rl_repo")

import numpy as np
import ml_dtypes

BF16 = ml_dtypes.bfloat16

B, S, D, H, E = 4, 2048, 1024, 4096, 8
K = 2
N_TOK = B * S            # 8192
CAP = (K * N_TOK) // E   # 2048 (capacity factor 1.0)
EPS = 1e-9
P = 128                  # SBUF partitions

_NC_CACHE = {}
_ROUTE_STATE = None


# --------------------------------------------------------------------------
# Host routing (replicates reference.py's gate exactly, in numpy fp32)
# --------------------------------------------------------------------------

def _route(xt, wg):
    """xt: [N, D] fp32, wg: [D, E] fp32 ->
    gidx [N,K] int, gvals [N,K] fp32 (keep-masked), pos [N,K] int, keep [N,K]"""
    logits = xt @ wg                                   # [N, E] fp32
    m = logits.max(axis=-1, keepdims=True)
    ex = np.exp(logits - m)
    scores = ex / ex.sum(axis=-1, keepdims=True)
    order = np.argsort(-scores, axis=1, kind="stable")  # jax top_k tie rule
    gidx = order[:, :K]                                 # [N, K]
    gvals = np.take_along_axis(scores, gidx, axis=1)
    gvals = gvals / np.clip(gvals.sum(-1, keepdims=True), EPS, None)

    n = xt.shape[0]
    offset = np.zeros(E, np.int64)
    pos = np.zeros((n, K), np.int64)
    keep = np.zeros((n, K), bool)
    rows = np.arange(n)
    for kk in range(K):
        ek = gidx[:, kk]
        oh = np.zeros((n, E), np.int64)
        oh[rows, ek] = 1
        loc = np.cumsum(oh, axis=0) - 1 + offset[None, :]
        offset = offset + oh.sum(axis=0)
        p = loc[rows, ek]
        kmask = p < CAP
        pos[:, kk] = np.where(kmask, p, 0)
        keep[:, kk] = kmask
    gvals = (gvals * keep).astype(np.float32)
    return gidx, gvals, pos, keep


# --------------------------------------------------------------------------
# Device kernel builder (one expert FFN per core, SPMD)
# --------------------------------------------------------------------------

def _build_nc(d, h, ntok, debug=False, act="Gelu", reps=1):
    """Expert FFN: y[ntok, d] = gelu(x[ntok, d] @ w1[d, h] + b1[h]) @ w2[h, d].

    Single-pass over weights; all matmul operands bf16; h resident in SBUF.

    Phase A (per h-tile m of 128): for each 512-token chunk tc, accumulate
    over the 8 D-tiles into one PSUM bank, then gelu(+b1) into the resident
    bf16 h[m] tile.
    Phase B (per output D-tile dt): for each 512-token chunk, accumulate over
    the 32 h-tiles into one PSUM bank, copy to bf16, DMA out as y^T.

    Device inputs (pre-laid-out on host for contiguous DMA):
      xt  : [d/P, P, ntok]        bf16  x^T tiles (partition = D)
      w1t : [h/P, P, d/P, P]      bf16  [m][dp][dt][hf] (lhsT layout)
      w2t : [d/P, P, h/P, P]      bf16  [dt][hp][m][df] (lhsT layout)
      b1t : [P, h/P]              f32   b1 transposed
    Output:
      yt  : [d/P, P, ntok]        bf16  y^T tiles (partition = D)
    """
    from concourse import bacc, mybir, tile

    dt_n = d // P            # 8  D tiles
    mt_n = h // P            # 32 H tiles
    tc_n = ntok // 512       # 4  512-token chunks
    assert ntok % 512 == 0

    f32 = mybir.dt.float32
    bf16 = mybir.dt.bfloat16
    actf = getattr(mybir.ActivationFunctionType, act)

    nc = bacc.Bacc("TRN2", target_bir_lowering=False, debug=debug,
                   enable_asserts=False, num_devices=1)

    xt_d = nc.dram_tensor("xt", [dt_n, P, ntok], bf16, kind="ExternalInput")
    w1_d = nc.dram_tensor("w1t", [mt_n, P, dt_n, P], bf16, kind="ExternalInput")
    w2_d = nc.dram_tensor("w2t", [dt_n, P, mt_n, P], bf16, kind="ExternalInput")
    b1_d = nc.dram_tensor("b1t", [P, mt_n], f32, kind="ExternalInput")
    y_d = nc.dram_tensor("yt", [dt_n, P, ntok], bf16, kind="ExternalOutput")

    with tile.TileContext(nc) as tc:
        with (
            tc.tile_pool(name="xpool", bufs=dt_n) as xpool,
            tc.tile_pool(name="cpool", bufs=2) as cpool,
            tc.tile_pool(name="w1pool", bufs=4) as w1pool,
            tc.tile_pool(name="hpool", bufs=mt_n + 2) as hpool,
            tc.tile_pool(name="w2pool", bufs=2) as w2pool,
            tc.tile_pool(name="ypool", bufs=4) as ypool,
            tc.tile_pool(name="psA", bufs=4, space="PSUM") as psA,
            tc.tile_pool(name="psB", bufs=3, space="PSUM") as psB,
            tc.tile_pool(name="psW", bufs=1, space="PSUM") as psW,
        ):
            b1_t = cpool.tile([P, mt_n], f32)
            nc.sync.dma_start(b1_t[:], b1_d[:])

            def prefetch_w1x(rep):
                """First w1 tiles + all x tiles for a rep.  Emitted before
                the previous rep's phase B so the scalar-queue issues land
                ahead of that rep's y stores (no rep-boundary input stall).
                x is split across both hwdge queues for startup bandwidth."""
                w1_pre = {}
                w1m = w1pool.tile([P, dt_n, P], bf16, tag="w1m",
                                  name=f"w1m_{rep}_0")
                nc.scalar.dma_start(w1m[:], w1_d[0])
                w1_pre[0] = w1m
                xg = [None] * dt_n
                for dti in range(dt_n):
                    xg_t = xpool.tile([P, ntok], bf16, tag="xg",
                                      name=f"xg_{rep}_{dti}")
                    eng = nc.sync if dti % 2 == 0 else nc.scalar
                    eng.dma_start(xg_t[:], xt_d[dti])
                    xg[dti] = xg_t
                for m in range(1, 4):
                    w1m = w1pool.tile([P, dt_n, P], bf16, tag="w1m",
                                      name=f"w1m_{rep}_{m}")
                    nc.scalar.dma_start(w1m[:], w1_d[m])
                    w1_pre[m] = w1m
                return w1_pre, xg

            def phase_a(rep, w1_pre, xg):
                """h[m] = gelu(sum_dt w1[dt][m].T @ x[dt] + b1[m])"""
                hs = []
                for m in range(mt_n):
                    if m in w1_pre:
                        w1m = w1_pre.pop(m)
                    else:
                        w1m = w1pool.tile([P, dt_n, P], bf16, tag="w1m",
                                          name=f"w1m_{rep}_{m}")
                        nc.scalar.dma_start(w1m[:], w1_d[m])
                    hm = hpool.tile([P, ntok], bf16, tag="hm",
                                    name=f"hm_{rep}_{m}")
                    for tci in range(tc_n):
                        c0, c1 = tci * 512, (tci + 1) * 512
                        ps = psA.tile([P, 512], f32, tag="psA")
                        for dti in range(dt_n):
                            nc.tensor.matmul(
                                ps[:],
                                w1m[:, dti, :],
                                xg[dti][:, c0:c1],
                                start=(dti == 0),
                                stop=(dti == dt_n - 1),
                            )
                        nc.scalar.activation(
                            hm[:, c0:c1], ps[:], actf,
                            bias=b1_t[:, m:m + 1], scale=1.0,
                        )
                    hs.append(hm)
                return hs

            def phase_b(rep, w2_pre, hs):
                """y^T[dt] = sum_m w2[dt][m].T @ h[m]"""
                for dti in range(dt_n):
                    if dti in w2_pre:
                        w2m = w2_pre.pop(dti)
                    else:
                        w2m = w2pool.tile([P, mt_n, P], bf16, tag="w2m",
                                          name=f"w2m_{rep}_{dti}")
                        nc.scalar.dma_start(w2m[:], w2_d[dti])
                    for tci in range(tc_n):
                        c0, c1 = tci * 512, (tci + 1) * 512
                        ps = psB.tile([P, 512], f32, tag="psB")
                        for m in range(mt_n):
                            nc.tensor.matmul(
                                ps[:],
                                w2m[:, m, :],
                                hs[m][:, c0:c1],
                                start=(m == 0),
                                stop=(m == mt_n - 1),
                            )
                        yt = ypool.tile([P, 512], bf16, tag="yt")
                        nc.vector.tensor_copy(yt[:], ps[:])
                        nc.scalar.dma_start(y_d[dti][:, c0:c1], yt[:])

            def prefetch_w2(rep):
                """First w2 tiles so phase B's first chains don't wait on a
                DMA issued after phase A's last gelu."""
                w2_pre = {}
                for dti in range(2):
                    w2m = w2pool.tile([P, mt_n, P], bf16, tag="w2m",
                                      name=f"w2m_{rep}_{dti}")
                    nc.scalar.dma_start(w2m[:], w2_d[dti])
                    w2_pre[dti] = w2m
                return w2_pre

            # PE p-state warmup: harmless 128-row matmuls on a zeroed tile
            # executed while the first x tiles stream in, so the real
            # matmuls start at full clock (ramp needs ~3us of busy PE).
            warm = cpool.tile([P, P], bf16, name="warm")
            nc.vector.memset(warm[:], 0)
            wps = psW.tile([P, P], f32, tag="warm")
            for _ in range(48):
                nc.tensor.matmul(wps[:], warm[:], warm[:], start=True,
                                 stop=True)

            w1_pre, xg = prefetch_w1x(0)
            w2_pre = prefetch_w2(0)
            for rep in range(reps):
                hs = phase_a(rep, w1_pre, xg)
                if rep + 1 < reps:
                    w1_pre, xg = prefetch_w1x(rep + 1)
                phase_b(rep, w2_pre, hs)
                if rep + 1 < reps:
                    w2_pre = prefetch_w2(rep + 1)

    nc.compile()
    return nc


def _get_nc(d, h, ntok, debug=False, reps=1):
    key = (d, h, ntok, debug, reps)
    if key not in _NC_CACHE:
        _NC_CACHE[key] = _build_nc(d, h, ntok, debug, reps=reps)
    return _NC_CACHE[key]


# --------------------------------------------------------------------------
# Host-side input layout per core
# --------------------------------------------------------------------------

def _core_inputs(disp_e, w1_e, w2_e, b1_e):
    """disp_e: [CAP, D], w1_e: [D, H], w2_e: [H, D], b1_e: [H] fp32."""
    xt = np.ascontiguousarray(disp_e.T.reshape(D // P, P, CAP)).astype(BF16)
    w1t = np.ascontiguousarray(
        w1_e.reshape(D // P, P, H // P, P).transpose(2, 1, 0, 3)).astype(BF16)
    w2t = np.ascontiguousarray(
        w2_e.reshape(H // P, P, D // P, P).transpose(2, 1, 0, 3)).astype(BF16)
    b1t = np.ascontiguousarray(b1_e.reshape(H // P, P).T)
    return {"xt": xt, "w1t": w1t, "w2t": w2t, "b1t": b1t}


def _get_runner(nc, n_cores):
    """Cached PJRT executable for an SPMD bass program (axon path of
    run_bass_kernel_spmd, with the jitted callable kept warm across calls)."""
    key = id(nc)
    if key in _NC_CACHE:
        return _NC_CACHE[key]

    import jax
    from jax.sharding import Mesh, PartitionSpec
    from jax.experimental.shard_map import shard_map
    from concourse import mybir
    from concourse.bass2jax import (_bass_exec_p, install_neuronx_cc_hook,
                                    partition_id_tensor)

    install_neuronx_cc_hook()

    partition_name = (nc.partition_id_tensor.name
                      if nc.partition_id_tensor else None)
    in_names, out_names, out_avals = [], [], []
    for alloc in nc.m.functions[0].allocations:
        if not isinstance(alloc, mybir.MemoryLocationSet):
            continue
        name = alloc.memorylocations[0].name
        if alloc.kind == "ExternalInput":
            if name != partition_name:
                in_names.append(name)
        elif alloc.kind == "ExternalOutput":
            out_names.append(name)
            shape = tuple(alloc.tensor_shape)
            out_avals.append(jax.core.ShapedArray(shape, mybir.dt.np(alloc.dtype)))
    n_params = len(in_names)
    n_outs = len(out_avals)
    in_names = in_names + out_names
    if partition_name is not None:
        in_names.append(partition_name)
    donate = tuple(range(n_params, n_params + n_outs))

    def _body(*args):
        operands = list(args)
        if partition_name is not None:
            operands.append(partition_id_tensor())
        outs = _bass_exec_p.bind(
            *operands,
            out_avals=tuple(out_avals),
            in_names=tuple(in_names),
            out_names=tuple(out_names),
            lowering_input_output_aliases=(),
            sim_require_finite=True,
            sim_require_nnan=True,
            nc=nc,
        )
        return tuple(outs)

    devices = jax.devices()[:n_cores]
    mesh = Mesh(np.asarray(devices), ("core",))
    in_specs = (PartitionSpec("core"),) * (n_params + n_outs)
    out_specs = (PartitionSpec("core"),) * n_outs
    sharded = jax.jit(
        shard_map(_body, mesh=mesh, in_specs=in_specs, out_specs=out_specs,
                  check_rep=False),
        donate_argnums=donate, keep_unused=True,
    )

    def run(in_maps, reps=1, time_reps=False):
        import time as _time
        concat_in = [
            np.concatenate([np.asarray(m[in_names[i]]) for m in in_maps], axis=0)
            for i in range(n_params)
        ]
        concat_in = [jax.device_put(a) for a in concat_in]
        zero_sets = []
        for _ in range(reps):
            zero_sets.append([
                jax.device_put(np.zeros((n_cores * av.shape[0], *av.shape[1:]),
                                        av.dtype))
                for av in out_avals
            ])
        for zs in zero_sets:
            for z in zs:
                z.block_until_ready()
        for a in concat_in:
            a.block_until_ready()
        times = []
        out_arrs = None
        for r in range(reps):
            t0 = _time.perf_counter()
            out_arrs = sharded(*concat_in, *zero_sets[r])
            for o in out_arrs:
                o.block_until_ready()
            times.append(_time.perf_counter() - t0)
        results = [
            {name: np.asarray(out_arrs[i]).reshape(n_cores, *out_avals[i].shape)[c]
             for i, name in enumerate(out_names)}
            for c in range(n_cores)
        ]
        if time_reps:
            return results, times
        return results

    _NC_CACHE[key] = run
    return run


def prepare(inputs, reps=1):
    """Routing + dispatch + per-core device input layout.  Returns
    (in_maps, nc); routing state is stashed on the module for finish()."""
    x = np.asarray(inputs["x"], np.float32)
    wg = np.asarray(inputs["wg"], np.float32)
    w1 = np.asarray(inputs["w1"], np.float32)
    b1 = np.asarray(inputs["b1"], np.float32)
    w2 = np.asarray(inputs["w2"], np.float32)

    xt = x.reshape(N_TOK, D)
    gidx, gvals, pos, keep = _route(xt, wg)
    global _ROUTE_STATE
    _ROUTE_STATE = (gidx, gvals, pos)

    # dispatch: slots are unique per expert, so assignment == scatter-add
    disp = np.zeros((E, CAP, D), np.float32)
    for kk in range(K):
        tok = np.nonzero(keep[:, kk])[0]
        disp[gidx[tok, kk], pos[tok, kk]] = xt[tok]

    in_maps = [_core_inputs(disp[e], w1[e], w2[e], b1[e]) for e in range(E)]
    nc = _get_nc(D, H, CAP, reps=reps)
    return in_maps, nc


def finish(inputs, results):
    """Combine: out = sum_k gvals * (y[e, pos] + b2[e])."""
    b2 = np.asarray(inputs["b2"], np.float32)
    gidx, gvals, pos = _ROUTE_STATE
    # yt: [d/P, P, ntok] bf16 y^T -> y [ntok, d] fp32
    y_all = np.stack([
        np.asarray(r["yt"], dtype=np.float32).reshape(D, CAP).T
        for r in results])  # [E,CAP,D]
    e_flat = gidx.reshape(-1)
    p_flat = pos.reshape(-1)
    yk = y_all[e_flat, p_flat] + b2[e_flat]
    w = gvals.reshape(-1).astype(np.float32)
    out = (yk * w[:, None]).reshape(N_TOK, K, D).sum(axis=1)
    return out.reshape(B, S, D).astype(np.float32)


def kernel(x, wg, w1, b1, w2, b2):
    inputs = {"x": x, "wg": wg, "w1": w1, "b1": b1, "w2": w2, "b2": b2}
    in_maps, nc = prepare(inputs)
    run = _get_runner(nc, E)
    results = run(in_maps)
    return finish(inputs, results)


# --------------------------------------------------------------------------
# Benchmarking helpers (test.py only)
# --------------------------------------------------------------------------

def bench(x, wg, w1, b1, w2, b2, reps=10):
    """Returns (reps1_times, reps5_times) per-call wall seconds for timing."""
    inputs = {"x": x, "wg": wg, "w1": w1, "b1": b1, "w2": w2, "b2": b2}
    in_maps, nc1 = prepare(inputs, reps=1)
    run1 = _get_runner(nc1, E)
    _, t1 = run1(in_maps, reps=reps, time_reps=True)

    nc5 = _get_nc(D, H, CAP, reps=5)
    run5 = _get_runner(nc5, E)
    _, t5 = run5(in_maps, reps=reps, time_reps=True)
    return t1, t5


# revision 29
# speedup vs baseline: 1.0020x; 1.0014x over previous
"""MoE (GPT MLP, top-2, GShard capacity) kernel for 8 Trainium2 NeuronCores.

Strategy (expert-parallel, matching the sharding hint):
  - Host: fp32 gate (softmax + top-2 + GShard capacity positions), dispatch
    gather.  Routing is O(N*E) int/scalar work - negligible next to the FFN -
    and the capacity scan is inherently sequential, so it runs on host.
  - Device: 8 cores, core e owns expert e.  Each core runs the expert FFN
    y = gelu(disp @ w1 + b1) @ w2 over its cap=2048 dispatched token slots.
    All matmuls in bf16 (inputs rounded host-side; PSUM accumulates fp32),
    which runs at the full PE rate with cheap (hideable) LDWEIGHTS and
    halves HBM traffic.  Weights stream exactly once: the whole 2048-token
    h activation stays resident in SBUF as bf16 (128 KB/partition).
  - Host: combine (gather + gate-weighted sum) + b2.

Self-contained: hardcodes B=4, S=2048, D=1024, H=4096, E=8, K=2, cap=2048.
"""

import sys

sys.path.insert(0, "/opt/trn_rl_repo")

import numpy as np
import ml_dtypes

BF16 = ml_dtypes.bfloat16

B, S, D, H, E = 4, 2048, 1024, 4096, 8
K = 2
N_TOK = B * S            # 8192
CAP = (K * N_TOK) // E   # 2048 (capacity factor 1.0)
EPS = 1e-9
P = 128                  # SBUF partitions

_NC_CACHE = {}
_ROUTE_STATE = None


# --------------------------------------------------------------------------
# Host routing (replicates reference.py's gate exactly, in numpy fp32)
# --------------------------------------------------------------------------

def _route(xt, wg):
    """xt: [N, D] fp32, wg: [D, E] fp32 ->
    gidx [N,K] int, gvals [N,K] fp32 (keep-masked), pos [N,K] int, keep [N,K]"""
    logits = xt @ wg                                   # [N, E] fp32
    m = logits.max(axis=-1, keepdims=True)
    ex = np.exp(logits - m)
    scores = ex / ex.sum(axis=-1, keepdims=True)
    order = np.argsort(-scores, axis=1, kind="stable")  # jax top_k tie rule
    gidx = order[:, :K]                                 # [N, K]
    gvals = np.take_along_axis(scores, gidx, axis=1)
    gvals = gvals / np.clip(gvals.sum(-1, keepdims=True), EPS, None)

    n = xt.shape[0]
    offset = np.zeros(E, np.int64)
    pos = np.zeros((n, K), np.int64)
    keep = np.zeros((n, K), bool)
    rows = np.arange(n)
    for kk in range(K):
        ek = gidx[:, kk]
        oh = np.zeros((n, E), np.int64)
        oh[rows, ek] = 1
        loc = np.cumsum(oh, axis=0) - 1 + offset[None, :]
        offset = offset + oh.sum(axis=0)
        p = loc[rows, ek]
        kmask = p < CAP
        pos[:, kk] = np.where(kmask, p, 0)
        keep[:, kk] = kmask
    gvals = (gvals * keep).astype(np.float32)
    return gidx, gvals, pos, keep


# --------------------------------------------------------------------------
# Device kernel builder (one expert FFN per core, SPMD)
# --------------------------------------------------------------------------

def _build_nc(d, h, ntok, debug=False, act="Gelu", reps=1):
    """Expert FFN: y[ntok, d] = gelu(x[ntok, d] @ w1[d, h] + b1[h]) @ w2[h, d].

    Single-pass over weights; all matmul operands bf16; h resident in SBUF.

    Phase A (per h-tile m of 128): for each 512-token chunk tc, accumulate
    over the 8 D-tiles into one PSUM bank, then gelu(+b1) into the resident
    bf16 h[m] tile.
    Phase B (per output D-tile dt): for each 512-token chunk, accumulate over
    the 32 h-tiles into one PSUM bank, copy to bf16, DMA out as y^T.

    Device inputs (pre-laid-out on host for contiguous DMA):
      xt  : [d/P, P, ntok]        bf16  x^T tiles (partition = D)
      w1t : [h/P, P, d/P, P]      bf16  [m][dp][dt][hf] (lhsT layout)
      w2t : [d/P, P, h/P, P]      bf16  [dt][hp][m][df] (lhsT layout)
      b1t : [P, h/P]              f32   b1 transposed
    Output:
      yt  : [d/P, P, ntok]        bf16  y^T tiles (partition = D)
    """
    from concourse import bacc, mybir, tile

    dt_n = d // P            # 8  D tiles
    mt_n = h // P            # 32 H tiles
    tc_n = ntok // 512       # 4  512-token chunks
    assert ntok % 512 == 0

    f32 = mybir.dt.float32
    bf16 = mybir.dt.bfloat16
    actf = getattr(mybir.ActivationFunctionType, act)

    nc = bacc.Bacc("TRN2", target_bir_lowering=False, debug=debug,
                   enable_asserts=False, num_devices=1)

    xt_d = nc.dram_tensor("xt", [dt_n, P, ntok], bf16, kind="ExternalInput")
    w1_d = nc.dram_tensor("w1t", [mt_n, P, dt_n, P], bf16, kind="ExternalInput")
    w2_d = nc.dram_tensor("w2t", [dt_n, P, mt_n, P], bf16, kind="ExternalInput")
    b1_d = nc.dram_tensor("b1t", [P, mt_n], f32, kind="ExternalInput")
    y_d = nc.dram_tensor("yt", [dt_n, P, ntok], bf16, kind="ExternalOutput")

    with tile.TileContext(nc) as tc:
        with (
            tc.tile_pool(name="xpool", bufs=dt_n) as xpool,
            tc.tile_pool(name="cpool", bufs=2) as cpool,
            tc.tile_pool(name="w1pool", bufs=4) as w1pool,
            tc.tile_pool(name="hpool", bufs=mt_n + 2) as hpool,
            tc.tile_pool(name="w2pool", bufs=2) as w2pool,
            tc.tile_pool(name="ypool", bufs=4) as ypool,
            tc.tile_pool(name="psA", bufs=4, space="PSUM") as psA,
            tc.tile_pool(name="psB", bufs=3, space="PSUM") as psB,
            tc.tile_pool(name="psW", bufs=1, space="PSUM") as psW,
        ):
            b1_t = cpool.tile([P, mt_n], f32)
            nc.sync.dma_start(b1_t[:], b1_d[:])

            def prefetch_w1x(rep):
                """First w1 tiles + all x tiles for a rep.  Emitted before
                the previous rep's phase B so the scalar-queue issues land
                ahead of that rep's y stores (no rep-boundary input stall).
                x is split across both hwdge queues for startup bandwidth."""
                w1_pre = {}
                w1m = w1pool.tile([P, dt_n, P], bf16, tag="w1m",
                                  name=f"w1m_{rep}_0")
                nc.scalar.dma_start(w1m[:], w1_d[0])
                w1_pre[0] = w1m
                xg = [None] * dt_n
                for dti in range(dt_n):
                    xg_t = xpool.tile([P, ntok], bf16, tag="xg",
                                      name=f"xg_{rep}_{dti}")
                    eng = nc.sync if dti % 2 == 0 else nc.scalar
                    eng.dma_start(xg_t[:], xt_d[dti])
                    xg[dti] = xg_t
                for m in range(1, 4):
                    w1m = w1pool.tile([P, dt_n, P], bf16, tag="w1m",
                                      name=f"w1m_{rep}_{m}")
                    nc.scalar.dma_start(w1m[:], w1_d[m])
                    w1_pre[m] = w1m
                return w1_pre, xg

            def phase_a(rep, w1_pre, xg):
                """h[m] = gelu(sum_dt w1[dt][m].T @ x[dt] + b1[m])"""
                hs = []
                for m in range(mt_n):
                    if m in w1_pre:
                        w1m = w1_pre.pop(m)
                    else:
                        w1m = w1pool.tile([P, dt_n, P], bf16, tag="w1m",
                                          name=f"w1m_{rep}_{m}")
                        nc.scalar.dma_start(w1m[:], w1_d[m])
                    hm = hpool.tile([P, ntok], bf16, tag="hm",
                                    name=f"hm_{rep}_{m}")
                    for tci in range(tc_n):
                        c0, c1 = tci * 512, (tci + 1) * 512
                        ps = psA.tile([P, 512], f32, tag="psA")
                        for dti in range(dt_n):
                            nc.tensor.matmul(
                                ps[:],
                                w1m[:, dti, :],
                                xg[dti][:, c0:c1],
                                start=(dti == 0),
                                stop=(dti == dt_n - 1),
                            )
                        nc.scalar.activation(
                            hm[:, c0:c1], ps[:], actf,
                            bias=b1_t[:, m:m + 1], scale=1.0,
                        )
                    hs.append(hm)
                return hs

            def phase_b(rep, w2_pre, hs):
                """y^T[dt] = sum_m w2[dt][m].T @ h[m]"""
                for dti in range(dt_n):
                    if dti in w2_pre:
                        w2m = w2_pre.pop(dti)
                    else:
                        w2m = w2pool.tile([P, mt_n, P], bf16, tag="w2m",
                                          name=f"w2m_{rep}_{dti}")
                        nc.scalar.dma_start(w2m[:], w2_d[dti])
                    for tci in range(tc_n):
                        c0, c1 = tci * 512, (tci + 1) * 512
                        ps = psB.tile([P, 512], f32, tag="psB")
                        for m in range(mt_n):
                            nc.tensor.matmul(
                                ps[:],
                                w2m[:, m, :],
                                hs[m][:, c0:c1],
                                start=(m == 0),
                                stop=(m == mt_n - 1),
                            )
                        yt = ypool.tile([P, 512], bf16, tag="yt")
                        nc.vector.tensor_copy(yt[:], ps[:])
                        nc.scalar.dma_start(y_d[dti][:, c0:c1], yt[:])

            def prefetch_w2(rep):
                """First w2 tiles so phase B's first chains don't wait on a
                DMA issued after phase A's last gelu."""
                w2_pre = {}
                for dti in range(2):
                    w2m = w2pool.tile([P, mt_n, P], bf16, tag="w2m",
                                      name=f"w2m_{rep}_{dti}")
                    nc.scalar.dma_start(w2m[:], w2_d[dti])
                    w2_pre[dti] = w2m
                return w2_pre

            # PE p-state warmup: harmless 128-row matmuls on a zeroed tile
            # executed while the first x tiles stream in, so the real
            # matmuls start at full clock (ramp needs ~3us of busy PE).
            warm = cpool.tile([P, P], bf16, name="warm")
            nc.vector.memset(warm[:], 0)
            wps = psW.tile([P, P], f32, tag="warm")
            for _ in range(48):
                nc.tensor.matmul(wps[:], warm[:], warm[:], start=True,
                                 stop=True)

            w1_pre, xg = prefetch_w1x(0)
            w2_pre = prefetch_w2(0)
            for rep in range(reps):
                hs = phase_a(rep, w1_pre, xg)
                if rep + 1 < reps:
                    w1_pre, xg = prefetch_w1x(rep + 1)
                phase_b(rep, w2_pre, hs)
                if rep + 1 < reps:
                    w2_pre = prefetch_w2(rep + 1)

    nc.compile()
    return nc


def _get_nc(d, h, ntok, debug=False, reps=1):
    key = (d, h, ntok, debug, reps)
    if key not in _NC_CACHE:
        _NC_CACHE[key] = _build_nc(d, h, ntok, debug, reps=reps)
    return _NC_CACHE[key]


# --------------------------------------------------------------------------
# Host-side input layout per core
# --------------------------------------------------------------------------

def _core_inputs(disp_e, w1_e, w2_e, b1_e):
    """disp_e: [CAP, D], w1_e: [D, H], w2_e: [H, D], b1_e: [H] fp32."""
    xt = np.ascontiguousarray(disp_e.T.reshape(D // P, P, CAP)).astype(BF16)
    w1t = np.ascontiguousarray(
        w1_e.reshape(D // P, P, H // P, P).transpose(2, 1, 0, 3)).astype(BF16)
    w2t = np.ascontiguousarray(
        w2_e.reshape(H // P, P, D // P, P).transpose(2, 1, 0, 3)).astype(BF16)
    b1t = np.ascontiguousarray(b1_e.reshape(H // P, P).T)
    return {"xt": xt, "w1t": w1t, "w2t": w2t, "b1t": b1t}


def _get_runner(nc, n_cores):
    """Cached PJRT executable for an SPMD bass program (axon path of
    run_bass_kernel_spmd, with the jitted callable kept warm across calls)."""
    key = id(nc)
    if key in _NC_CACHE:
        return _NC_CACHE[key]

    import jax
    from jax.sharding import Mesh, PartitionSpec
    from jax.experimental.shard_map import shard_map
    from concourse import mybir
    from concourse.bass2jax import (_bass_exec_p, install_neuronx_cc_hook,
                                    partition_id_tensor)

    install_neuronx_cc_hook()

    partition_name = (nc.partition_id_tensor.name
                      if nc.partition_id_tensor else None)
    in_names, out_names, out_avals = [], [], []
    for alloc in nc.m.functions[0].allocations:
        if not isinstance(alloc, mybir.MemoryLocationSet):
            continue
        name = alloc.memorylocations[0].name
        if alloc.kind == "ExternalInput":
            if name != partition_name:
                in_names.append(name)
        elif alloc.kind == "ExternalOutput":
            out_names.append(name)
            shape = tuple(alloc.tensor_shape)
            out_avals.append(jax.core.ShapedArray(shape, mybir.dt.np(alloc.dtype)))
    n_params = len(in_names)
    n_outs = len(out_avals)
    in_names = in_names + out_names
    if partition_name is not None:
        in_names.append(partition_name)
    donate = tuple(range(n_params, n_params + n_outs))

    def _body(*args):
        operands = list(args)
        if partition_name is not None:
            operands.append(partition_id_tensor())
        outs = _bass_exec_p.bind(
            *operands,
            out_avals=tuple(out_avals),
            in_names=tuple(in_names),
            out_names=tuple(out_names),
            lowering_input_output_aliases=(),
            sim_require_finite=True,
            sim_require_nnan=True,
            nc=nc,
        )
        return tuple(outs)

    devices = jax.devices()[:n_cores]
    mesh = Mesh(np.asarray(devices), ("core",))
    in_specs = (PartitionSpec("core"),) * (n_params + n_outs)
    out_specs = (PartitionSpec("core"),) * n_outs
    sharded = jax.jit(
        shard_map(_body, mesh=mesh, in_specs=in_specs, out_specs=out_specs,
                  check_rep=False),
        donate_argnums=donate, keep_unused=True,
    )

    def run(in_maps, reps=1, time_reps=False):
        import time as _time
        concat_in = [
            np.concatenate([np.asarray(m[in_names[i]]) for m in in_maps], axis=0)
            for i in range(n_params)
        ]
        concat_in = [jax.device_put(a) for a in concat_in]
        zero_sets = []
        for _ in range(reps):
            zero_sets.append([
                jax.device_put(np.zeros((n_cores * av.shape[0], *av.shape[1:]),
                                        av.dtype))
                for av in out_avals
            ])
        for zs in zero_sets:
            for z in zs:
                z.block_until_ready()
        for a in concat_in:
            a.block_until_ready()
        times = []
        out_arrs = None
        for r in range(reps):
            t0 = _time.perf_counter()
            out_arrs = sharded(*concat_in, *zero_sets[r])
            for o in out_arrs:
                o.block_until_ready()
            times.append(_time.perf_counter() - t0)
        results = [
            {name: np.asarray(out_arrs[i]).reshape(n_cores, *out_avals[i].shape)[c]
             for i, name in enumerate(out_names)}
            for c in range(n_cores)
        ]
        if time_reps:
            return results, times
        return results

    _NC_CACHE[key] = run
    return run


def prepare(inputs, reps=1):
    """Routing + dispatch + per-core device input layout.  Returns
    (in_maps, nc); routing state is stashed on the module for finish()."""
    x = np.asarray(inputs["x"], np.float32)
    wg = np.asarray(inputs["wg"], np.float32)
    w1 = np.asarray(inputs["w1"], np.float32)
    b1 = np.asarray(inputs["b1"], np.float32)
    w2 = np.asarray(inputs["w2"], np.float32)

    xt = x.reshape(N_TOK, D)
    gidx, gvals, pos, keep = _route(xt, wg)
    global _ROUTE_STATE
    _ROUTE_STATE = (gidx, gvals, pos)

    # dispatch: slots are unique per expert, so assignment == scatter-add
    disp = np.zeros((E, CAP, D), np.float32)
    for kk in range(K):
        tok = np.nonzero(keep[:, kk])[0]
        disp[gidx[tok, kk], pos[tok, kk]] = xt[tok]

    in_maps = [_core_inputs(disp[e], w1[e], w2[e], b1[e]) for e in range(E)]
    nc = _get_nc(D, H, CAP, reps=reps)
    return in_maps, nc


def finish(inputs, results):
    """Combine: out = sum_k gvals * (y[e, pos] + b2[e])."""
    b2 = np.asarray(inputs["b2"], np.float32)
    gidx, gvals, pos = _ROUTE_STATE
    # yt: [d/P, P, ntok] bf16 y^T -> y [ntok, d] fp32
    y_all = np.stack([
        np.asarray(r["yt"], dtype=np.float32).reshape(D, CAP).T
        for r in results])  # [E,CAP,D]
    e_flat = gidx.reshape(-1)
    p_flat = pos.reshape(-1)
    yk = y_all[e_flat, p_flat] + b2[e_flat]
    w = gvals.reshape(-1).astype(np.float32)
    out = (yk * w[:, None]).reshape(N_TOK, K, D).sum(axis=1)
    return out.reshape(B, S, D).astype(np.float32)


def kernel(x, wg, w1, b1, w2, b2):
    inputs = {"x": x, "wg": wg, "w1": w1, "b1": b1, "w2": w2, "b2": b2}
    in_maps, nc = prepare(inputs)
    run = _get_runner(nc, E)
    results = run(in_maps)
    return finish(inputs, results)


# --------------------------------------------------------------------------
# Benchmarking helpers (test.py only)
# --------------------------------------------------------------------------

def bench(x, wg, w1, b1, w2, b2, reps=10):
    """Returns (reps1_times, reps5_times) per-call wall seconds for timing."""
    inputs = {"x": x, "wg": wg, "w1": w1, "b1": b1, "w2": w2, "b2": b2}
    in_maps, nc1 = prepare(inputs, reps=1)
    run1 = _get_runner(nc1, E)
    _, t1 = run1(in_maps, reps=reps, time_reps=True)

    nc5 = _get_nc(D, H, CAP, reps=5)
    run5 = _get_runner(nc5, E)
    _, t5 = run5(in_maps, reps=reps, time_reps=True)
    return t1, t5
